# Initial kernel scaffold
#
"""Multi-head latent attention (MLA) kernel for Trainium2, 8-core SPMD.

Sharding: tensor-parallel over heads. Core c owns global heads {2c, 2c+1}
for both batch elements, i.e. the contiguous slice x[:, :, 128c:128(c+1)].
No collectives: the host slices inputs per core and concatenates outputs.

On-chip algorithm (per core, per (b, head) pair), everything feature-major
([feature, t] layout) so no transposes are needed in the attention core:

  - Host folds the latent projections:  C_q = W_UQ @ W_DQ  (64x64),
    C_qr = W_QR @ diag(q_norm_w) @ C_q, C_kv = W_UKV @ W_DKV (128x64),
    with the norm weights folded into the content rows. One matmul
    produces [w*q_c; q_r*rms] rows, one [w*k_c; k_r], one v -- all
    directly from x^T.
  - RMSNorm becomes a per-column (per-t) scale applied after the fact:
      rs = 8*(ssq + 64*eps)^-1/2 = exp(-0.5*ln(ssq + 64*eps) + ln 8)
    computed for q and k in ONE Ln + ONE Exp over a strided partition AP
    (rows 0 and 32 of one PSUM bank), then broadcast across partitions by
    GPSIMD partition_broadcast (the otherwise-idle engine).  ssq itself is
    a ones-vector matmul over ACT-squared rows (Square's scale input
    un-folds the norm weight so the norm sees raw q_c).
  - Scores are computed transposed, S^T[tk, tq] = k^T . q, softmax without
    max-subtraction (|scores| <~ 8 so exp is safe in fp32), denominator via
    a ones-column appended to v in the ctx matmul.
  - The softmax denominator is applied per-t AFTER the final PE transpose
    (t becomes the partition axis there) as a tensor_scalar multiply.
  - Causality at 128-col granularity: off-diagonal [128,512] tiles computed
    full, diagonal tiles narrowed + masked with a host triangle mask.
  - All matmuls run as float32r (full PE rate for moving dim >= 256).
  - All ACT functions used (Exp, Ln, Square, Copy) live in the single
    'natural_log_exp_and_others' table set; the set-selection table is
    patched so the first-fit chooser can only pick that set (otherwise it
    alternates exp_and_others <-> natural_log, ~1.3us per switch).
"""

import sys

if "/opt/trn_rl_repo" not in sys.path:
    sys.path.insert(0, "/opt/trn_rl_repo")

import math
from contextlib import ExitStack

import numpy as np

import concourse.bass as bass
import concourse.mybir as mybir
import concourse.tile as tile
from concourse import bacc

B, T, D = 2, 2048, 1024
H, HD, DL, NR = 16, 64, 512, 64
EPS = 1.1920929e-07
NCORES = 8
TT = 512            # tq tile width
NJ = T // TT        # tq tiles per (b, h) pair
NK = T // 128       # tk chunks
AF = mybir.ActivationFunctionType
F32 = mybir.dt.float32
F32R = mybir.dt.float32r


def _r(ap):
    """float32r view of an fp32 AP (unused; tiles are typed F32R directly)."""
    return ap.bitcast(F32R)


def _patch_act_tables():
    """Restrict the ACT table-set chooser to natural_log_exp_and_others.

    All activation funcs this kernel uses (Exp, Ln, Square, Copy) are in
    that one set, but bacc's first-fit chooser would otherwise alternate
    between exp_and_others (Exp) and natural_log (Ln) per instruction --
    64 table loads at ~1.3us each.  Emptying every other set (order and
    hence act_func_set_id indices preserved) forces a single load.
    """
    import functools

    import concourse.hw_specs as hw_specs

    orig = hw_specs.get_activation_tables.__wrapped__
    keep = "natural_log_exp_and_others"

    @functools.cache
    def patched(module_arch):
        t = orig(module_arch)
        return {k: (v if k == keep else set()) for k, v in t.items()}

    hw_specs.get_activation_tables = patched
    bacc.get_activation_tables = patched


def build_program(reps=1):
    _patch_act_tables()
    nc = bacc.Bacc(
        "TRN2", target_bir_lowering=False, debug=False, enable_asserts=False
    )
    dt = nc.dram_tensor
    xs = dt("xs", [B, T, 128], F32, kind="ExternalInput").ap()
    wq = dt("wq", [HD, 128], F32R, kind="ExternalInput").ap()
    wk = dt("wk", [HD, 128], F32R, kind="ExternalInput").ap()
    wv = dt("wv", [HD, HD], F32R, kind="ExternalInput").ap()
    wo = dt("wo", [HD, HD], F32R, kind="ExternalInput").ap()
    w2q = dt("w2q", [HD, 1], F32R, kind="ExternalInput").ap()
    w2k = dt("w2k", [HD, 1], F32R, kind="ExternalInput").ap()
    ident = dt("ident", [128, 128], F32, kind="ExternalInput").ap()
    gm = dt("gm", [128, 256], F32, kind="ExternalInput").ap()
    vone = dt("vone", [128, NK], F32R, kind="ExternalInput").ap()
    rtag = dt("rtag", [1, reps], F32, kind="ExternalInput").ap()
    out = dt("out", [B, T, 128], F32, kind="ExternalOutput").ap()

    with tile.TileContext(nc) as tc, ExitStack() as ctx:
        pool = ctx.enter_context(tc.tile_pool(name="sb", bufs=1))
        ppool = ctx.enter_context(tc.tile_pool(name="ps", bufs=1, space="PSUM"))

        def sb(shape, tag, bufs=1, dt=F32):
            return pool.tile(shape, dt, tag=tag, bufs=bufs, name=tag)

        def ps(shape, tag, bufs=1):
            return ppool.tile(shape, F32, tag=tag, bufs=bufs, name=tag)

        # ---- constants to SBUF ----
        wq_s = sb([HD, 128], "wq", dt=F32R)
        nc.sync.dma_start(wq_s, wq)
        wk_s = sb([HD, 128], "wk", dt=F32R)
        nc.sync.dma_start(wk_s, wk)
        wv_s = sb([HD, HD], "wv", dt=F32R)
        nc.sync.dma_start(wv_s, wv)
        wo_s = sb([HD, HD], "wo", dt=F32R)
        nc.sync.dma_start(wo_s, wo)
        w2q_s = sb([HD, 1], "w2q", dt=F32R)
        nc.sync.dma_start(w2q_s, w2q)
        w2k_s = sb([HD, 1], "w2k", dt=F32R)
        nc.sync.dma_start(w2k_s, w2k)
        ident_s = sb([128, 128], "ident")
        nc.sync.dma_start(ident_s, ident)
        gm_s = sb([128, 256], "gm")
        nc.sync.dma_start(gm_s, gm)
        one_at64 = gm_s[64:65, 255:256]    # [1, 1] ones at partition 64
        # bias tiles for the strided-partition Ln/Exp (rows 0 and 32)
        rtag_s = sb([1, reps], "rtag")
        nc.sync.dma_start(rtag_s, rtag)
        epsb = sb([1, 1], "epsb")
        nc.gpsimd.memset(epsb, float(HD * EPS))
        ln8b = sb([1, 1], "ln8b")
        nc.gpsimd.memset(ln8b, float(math.log(8.0)))

        for _rep in range(reps):
            # ---- x -> feature-major xT[b][hh] = x[b, :, hh*64:(hh+1)*64]^T ----
            xT = [[sb([HD, T], f"xT{b}{hh}", dt=F32R) for hh in range(2)] for b in range(B)]
            for b in range(B):
                for c in range(NK):
                    xn = sb([128, 128], "xn", bufs=4)
                    nc.sync.dma_start(xn, xs[b, c * 128:(c + 1) * 128, :])
                    for hh in range(2):
                        pt = ps([HD, 128], "paux")
                        nc.tensor.transpose(
                            pt, xn[:, hh * HD:(hh + 1) * HD], ident_s
                        )
                        nc.vector.tensor_copy(xT[b][hh][:, c * 128:(c + 1) * 128], pt)

            obuf = [sb([128, T], f"ob{b}") for b in range(B)]

            # Software-pipelined emission: A(pair p+1) is emitted before
            # B(pair p) so the serial pair-wide rs chain (ssq -> Ln/Exp
            # -> transpose -> flatten DMA -> broadcast -> in-place muls)
            # hides under the previous pair's attention phase.
            st = {}

            def emit_A(b, hh):
                xh = xT[b][hh]
                qT = sb([128, T], "qT", bufs=2, dt=F32R)
                kT = sb([128, T], "kT", bufs=2, dt=F32R)
                vt = sb([128, NK * 65], "vt", bufs=2, dt=F32R)
                nc.sync.dma_start(vt[:, 64::65], vone)

                # ---------- phase A: projections + norm ----------
                for j in range(NJ):
                    tsl = slice(j * TT, (j + 1) * TT)
                    pq = ps([128, TT], "pq")
                    nc.tensor.matmul(pq, wq_s, xh[:, tsl],
                                     start=True, stop=True)
                    pk = ps([128, TT], "pk")
                    nc.tensor.matmul(pk, wk_s, xh[:, tsl],
                                     start=True, stop=True)

                    # ssq for q (row 0) and k (row 32) of one PSUM bank;
                    # the winv^2 norm-weight unfold rides in the lhsT column
                    sq = sb([HD, TT], "sq", bufs=3, dt=F32R)
                    nc.scalar.square(sq, pq[0:HD, :])
                    sqk = sb([HD, TT], "sq", bufs=3, dt=F32R)
                    nc.scalar.square(sqk, pk[0:HD, :])
                    pssq = ps([1, TT], "pss", bufs=2)
                    nc.tensor.matmul(pssq, w2q_s, sq, start=True, stop=True)
                    pssk = ps([1, TT], "pss", bufs=2)
                    nc.tensor.matmul(pssk, w2k_s, sqk, start=True, stop=True)
                    lnq = sb([1, TT], "lnb", bufs=3)
                    nc.scalar.activation(lnq, pssq, AF.Ln, bias=epsb)
                    lnk = sb([1, TT], "lnb", bufs=3)
                    nc.scalar.activation(lnk, pssk, AF.Ln, bias=epsb)
                    rsq = sb([1, TT], "rs2", bufs=3)
                    nc.scalar.activation(rsq, lnq, AF.Exp, scale=-0.5,
                                         bias=ln8b)
                    rsk = sb([1, TT], "rs2", bufs=3)
                    nc.scalar.activation(rsk, lnk, AF.Exp, scale=-0.5,
                                         bias=ln8b)
                    bqb = sb([128, TT], "bqb", bufs=3)
                    nc.gpsimd.partition_broadcast(bqb, rsq)
                    nc.vector.tensor_mul(qT[:, tsl], pq, bqb)
                    bkb = sb([128, TT], "bkb", bufs=3)
                    nc.gpsimd.memset(bkb[HD:128, :], 1.0)
                    nc.gpsimd.partition_broadcast(bkb[0:HD, :], rsk)
                    nc.vector.tensor_mul(kT[:, tsl], pk, bkb)

                    # v computed directly in [t, d] layout (x^T as lhsT)
                    for u in range(TT // 128):
                        w = j * (TT // 128) + u
                        pvt = ps([128, HD], "paux")
                        nc.tensor.matmul(
                            pvt, xh[:, w * 128:(w + 1) * 128], wv_s,
                            start=True, stop=True,
                        )
                        nc.scalar.copy(vt[:, w * 65:w * 65 + HD], pvt)

                st[(b, hh)] = (qT, kT, vt)

            def emit_B(b, hh):
                qT, kT, vt = st.pop((b, hh))
                # ---------- phase B: attention ----------
                for j in range(NJ):
                    pctx = ps([65, TT], "pctx")
                    nmax = 4 * j + 3
                    for i in range(nmax + 1):
                        p = i - 4 * j
                        if p < 1:
                            off, w_ = 0, 512
                        elif p == 1:
                            off, w_ = 128, 384
                        else:
                            off, w_ = 256, 256
                        psc = ps([128, w_], "psc", bufs=2)
                        nc.tensor.matmul(
                            psc, kT[:, i * 128:(i + 1) * 128],
                            qT[:, j * TT + off:j * TT + off + w_],
                            start=True, stop=True,
                        )
                        E = sb([128, w_], "E", bufs=6, dt=F32R)
                        nc.scalar.activation(E, psc, AF.Exp, scale=0.125)
                        if p >= 0:
                            if p <= 2:
                                nc.vector.tensor_mul(
                                    E[:, 0:128], E[:, 0:128], gm_s[:, 128:256]
                                )
                            else:
                                nc.vector.tensor_mul(
                                    E[:, 0:256], E[:, 0:256], gm_s[:, 0:256]
                                )
                        nc.tensor.matmul(
                            pctx[:, off:off + w_],
                            vt[:, i * 65:(i + 1) * 65], E,
                            start=(i == 0), stop=(i == nmax),
                        )

                    # ---- unit tail: W_O in [t, o] layout, per-t 1/D ----
                    ctxs = sb([HD, TT], "ctxs", bufs=3, dt=F32R)
                    nc.vector.tensor_copy(ctxs, pctx[0:HD, :])
                    dsb = sb([65, TT], "dsb", bufs=3)
                    nc.vector.reciprocal(dsb[64:65, :], pctx[64:65, :])
                    for c in range(TT // 128):
                        # 1/D as a column: [1,128] row -> [128,1] via PE
                        pdv = ps([128, 1], "paux")
                        nc.tensor.transpose(
                            pdv, dsb[64:65, c * 128:(c + 1) * 128],
                            one_at64,
                        )
                        dvc = sb([128, 1], "dvc", bufs=4)
                        nc.vector.tensor_copy(dvc, pdv)
                        pt2 = ps([128, HD], "paux")
                        nc.tensor.matmul(
                            pt2, ctxs[:, c * 128:(c + 1) * 128], wo_s,
                            start=True, stop=True,
                        )
                        tc_ = j * (TT // 128) + c
                        nc.vector.tensor_scalar_mul(
                            obuf[b][:, tc_ * 128 + hh * HD:
                                    tc_ * 128 + hh * HD + HD],
                            pt2, dvc,
                        )

                if hh == 1:
                    # both heads of this b done -> store
                    nc.sync.dma_start(
                        out[b].rearrange("(a p) o -> p a o", p=128),
                        obuf[b].rearrange("p (a o) -> p a o", o=128),
                    )

            pairs = [(bb, hh) for bb in range(B) for hh in range(2)]
            for i in range(len(pairs) + 1):
                if i < len(pairs):
                    emit_A(*pairs[i])
                if i > 0:
                    emit_B(*pairs[i - 1])

    nc.compile()
    return nc


_CACHE = {}


def get_program(reps=1):
    key = f"nc{reps}"
    if key not in _CACHE:
        _CACHE[key] = build_program(reps)
    return _CACHE[key]


def prep_consts(W_DQ, W_UQ, W_DKV, W_UKV, W_QR, W_KR, W_O, q_norm_w, k_norm_w):
    f8 = np.float64
    wqn = q_norm_w.astype(f8)
    wkn = k_norm_w.astype(f8)
    C_q = W_UQ.astype(f8) @ W_DQ.astype(f8)                    # (64e, 64d)
    C_qr = W_QR.astype(f8) @ (wqn[:, None] * C_q)
    wq = np.ascontiguousarray(
        np.concatenate([C_q.T * wqn[None, :], C_qr.T], axis=1)
    ).astype(np.float32)
    C_kv = W_UKV.astype(f8) @ W_DKV.astype(f8)                 # (128, 64)
    wk = np.ascontiguousarray(
        np.concatenate([C_kv[:HD].T * wkn[None, :], W_KR.T.astype(f8)],
                       axis=1)
    ).astype(np.float32)
    wv = np.ascontiguousarray(C_kv[HD:].T).astype(np.float32)
    wo = np.ascontiguousarray(W_O.T).astype(np.float32)
    # ssq lhsT columns carry 1/w^2 so the norm sees raw q_c sums.
    # (a norm weight of exactly 0 would need a different path; ones here.)
    winvq = np.where(wqn != 0, 1.0 / np.where(wqn != 0, wqn, 1.0) ** 2, 0.0)
    winvk = np.where(wkn != 0, 1.0 / np.where(wkn != 0, wkn, 1.0) ** 2, 0.0)
    ident = np.eye(128, dtype=np.float32)
    gm = (np.arange(256)[None, :] - 128 >= np.arange(128)[:, None]).astype(
        np.float32
    )
    return dict(
        wq=wq, wk=wk, wv=wv, wo=wo,
        w2q=winvq.reshape(HD, 1).astype(np.float32),
        w2k=winvk.reshape(HD, 1).astype(np.float32),
        ident=ident, gm=gm,
        vone=np.ones((128, NK), np.float32),
    )


def make_in_maps(inputs, reps=1):
    x = np.asarray(inputs["x"], np.float32)
    consts = prep_consts(
        *(np.asarray(inputs[k], np.float32) for k in (
            "W_DQ", "W_UQ", "W_DKV", "W_UKV", "W_QR", "W_KR", "W_O",
            "q_norm_w", "k_norm_w"))
    )
    in_maps = []
    for c in range(NCORES):
        m = dict(consts)
        m["xs"] = np.ascontiguousarray(x[:, :, c * 128:(c + 1) * 128])
        m["rtag"] = np.zeros((1, reps), np.float32)
        in_maps.append(m)
    return in_maps


def kernel(**inputs):
    from concourse.bass_utils import run_bass_kernel_spmd

    nc = get_program()
    in_maps = make_in_maps(inputs)
    res = run_bass_kernel_spmd(nc, in_maps, core_ids=list(range(NCORES)))
    out = np.empty((B, T, D), np.float32)
    for c in range(NCORES):
        out[:, :, c * 128:(c + 1) * 128] = res.results[c]["out"]
    return out



# revision 1
# speedup vs baseline: 15.0523x; 15.0523x over previous
"""Multi-head latent attention (MLA) kernel for Trainium2, 8-core SPMD.

Sharding: tensor-parallel over heads. Core c owns global heads {2c, 2c+1}
for both batch elements, i.e. the contiguous slice x[:, :, 128c:128(c+1)].
No collectives: the host slices inputs per core and concatenates outputs.

On-chip algorithm (per core, per (b, head) pair), everything feature-major
([feature, t] layout) so no transposes are needed in the attention core:

  - Host folds the latent projections:  C_q = W_UQ @ W_DQ  (64x64),
    C_qr = W_QR @ diag(q_norm_w) @ C_q, C_kv = W_UKV @ W_DKV (128x64),
    with the norm weights folded into the content rows. One matmul
    produces [w*q_c; q_r*rms] rows, one [w*k_c; k_r], one v -- all
    directly from x^T.
  - RMSNorm becomes a per-column (per-t) scale applied after the fact:
      rs = 8*(ssq + 64*eps)^-1/2 = exp(-0.5*ln(ssq + 64*eps) + ln 8)
    computed for q and k in ONE Ln + ONE Exp over a strided partition AP
    (rows 0 and 32 of one PSUM bank), then broadcast across partitions by
    GPSIMD partition_broadcast (the otherwise-idle engine).  ssq itself is
    a ones-vector matmul over ACT-squared rows (Square's scale input
    un-folds the norm weight so the norm sees raw q_c).
  - Scores are computed transposed, S^T[tk, tq] = k^T . q, softmax without
    max-subtraction (|scores| <~ 8 so exp is safe in fp32), denominator via
    a ones-column appended to v in the ctx matmul.
  - The softmax denominator is applied per-t AFTER the final PE transpose
    (t becomes the partition axis there) as a tensor_scalar multiply.
  - Causality at 128-col granularity: off-diagonal [128,512] tiles computed
    full, diagonal tiles narrowed + masked with a host triangle mask.
  - All matmuls run as float32r (full PE rate for moving dim >= 256).
  - All ACT functions used (Exp, Ln, Square, Copy) live in the single
    'natural_log_exp_and_others' table set; the set-selection table is
    patched so the first-fit chooser can only pick that set (otherwise it
    alternates exp_and_others <-> natural_log, ~1.3us per switch).
"""

import sys

if "/opt/trn_rl_repo" not in sys.path:
    sys.path.insert(0, "/opt/trn_rl_repo")

import math
from contextlib import ExitStack

import numpy as np

import concourse.bass as bass
import concourse.mybir as mybir
import concourse.tile as tile
from concourse import bacc

B, T, D = 2, 2048, 1024
H, HD, DL, NR = 16, 64, 512, 64
EPS = 1.1920929e-07
NCORES = 8
TT = 512            # tq tile width
NJ = T // TT        # tq tiles per (b, h) pair
NK = T // 128       # tk chunks
AF = mybir.ActivationFunctionType
F32 = mybir.dt.float32
F32R = mybir.dt.float32r


def _r(ap):
    """float32r view of an fp32 AP (unused; tiles are typed F32R directly)."""
    return ap.bitcast(F32R)


def _patch_act_tables():
    """Restrict the ACT table-set chooser to natural_log_exp_and_others.

    All activation funcs this kernel uses (Exp, Ln, Square, Copy) are in
    that one set, but bacc's first-fit chooser would otherwise alternate
    between exp_and_others (Exp) and natural_log (Ln) per instruction --
    64 table loads at ~1.3us each.  Emptying every other set (order and
    hence act_func_set_id indices preserved) forces a single load.
    """
    import functools

    import concourse.hw_specs as hw_specs

    orig = hw_specs.get_activation_tables.__wrapped__
    keep = "natural_log_exp_and_others"

    @functools.cache
    def patched(module_arch):
        t = orig(module_arch)
        return {k: (v if k == keep else set()) for k, v in t.items()}

    hw_specs.get_activation_tables = patched
    bacc.get_activation_tables = patched


def build_program(reps=1):
    _patch_act_tables()
    nc = bacc.Bacc(
        "TRN2", target_bir_lowering=False, debug=False, enable_asserts=False
    )
    dt = nc.dram_tensor
    xs = dt("xs", [B, T, 128], F32, kind="ExternalInput").ap()
    wq = dt("wq", [HD, 128], F32R, kind="ExternalInput").ap()
    wk = dt("wk", [HD, 128], F32R, kind="ExternalInput").ap()
    wv = dt("wv", [HD, HD], F32R, kind="ExternalInput").ap()
    wo = dt("wo", [HD, HD], F32R, kind="ExternalInput").ap()
    w2q = dt("w2q", [HD, 1], F32R, kind="ExternalInput").ap()
    w2k = dt("w2k", [HD, 1], F32R, kind="ExternalInput").ap()
    ident = dt("ident", [128, 128], F32, kind="ExternalInput").ap()
    gm = dt("gm", [128, 256], F32, kind="ExternalInput").ap()
    vone = dt("vone", [128, NK], F32R, kind="ExternalInput").ap()
    rtag = dt("rtag", [1, reps], F32, kind="ExternalInput").ap()
    out = dt("out", [B, T, 128], F32, kind="ExternalOutput").ap()

    with tile.TileContext(nc) as tc, ExitStack() as ctx:
        pool = ctx.enter_context(tc.tile_pool(name="sb", bufs=1))
        ppool = ctx.enter_context(tc.tile_pool(name="ps", bufs=1, space="PSUM"))

        def sb(shape, tag, bufs=1, dt=F32):
            return pool.tile(shape, dt, tag=tag, bufs=bufs, name=tag)

        def ps(shape, tag, bufs=1):
            return ppool.tile(shape, F32, tag=tag, bufs=bufs, name=tag)

        # ---- constants to SBUF ----
        wq_s = sb([HD, 128], "wq", dt=F32R)
        nc.sync.dma_start(wq_s, wq)
        wk_s = sb([HD, 128], "wk", dt=F32R)
        nc.sync.dma_start(wk_s, wk)
        wv_s = sb([HD, HD], "wv", dt=F32R)
        nc.sync.dma_start(wv_s, wv)
        wo_s = sb([HD, HD], "wo", dt=F32R)
        nc.sync.dma_start(wo_s, wo)
        w2q_s = sb([HD, 1], "w2q", dt=F32R)
        nc.sync.dma_start(w2q_s, w2q)
        w2k_s = sb([HD, 1], "w2k", dt=F32R)
        nc.sync.dma_start(w2k_s, w2k)
        ident_s = sb([128, 128], "ident")
        nc.sync.dma_start(ident_s, ident)
        gm_s = sb([128, 256], "gm")
        nc.sync.dma_start(gm_s, gm)
        one_at64 = gm_s[64:65, 255:256]    # [1, 1] ones at partition 64
        # bias tiles for the strided-partition Ln/Exp (rows 0 and 32)
        rtag_s = sb([1, reps], "rtag")
        nc.sync.dma_start(rtag_s, rtag)
        epsb = sb([1, 1], "epsb")
        nc.gpsimd.memset(epsb, float(HD * EPS))
        ln8b = sb([1, 1], "ln8b")
        nc.gpsimd.memset(ln8b, float(math.log(8.0)))

        for _rep in range(reps):
            # ---- x -> feature-major xT[b][hh] = x[b, :, hh*64:(hh+1)*64]^T ----
            xT = [[sb([HD, T], f"xT{b}{hh}", dt=F32R) for hh in range(2)] for b in range(B)]
            for b in range(B):
                for c in range(NK):
                    xn = sb([128, 128], "xn", bufs=4)
                    nc.sync.dma_start(xn, xs[b, c * 128:(c + 1) * 128, :])
                    for hh in range(2):
                        pt = ps([HD, 128], "paux")
                        nc.tensor.transpose(
                            pt, xn[:, hh * HD:(hh + 1) * HD], ident_s
                        )
                        nc.vector.tensor_copy(xT[b][hh][:, c * 128:(c + 1) * 128], pt)

            obuf = [sb([128, T], f"ob{b}") for b in range(B)]

            # Software-pipelined emission: A(pair p+1) is emitted before
            # B(pair p) so the serial pair-wide rs chain (ssq -> Ln/Exp
            # -> transpose -> flatten DMA -> broadcast -> in-place muls)
            # hides under the previous pair's attention phase.
            st = {}

            def emit_A(b, hh):
                xh = xT[b][hh]
                qT = sb([128, T], "qT", bufs=2, dt=F32R)
                kT = sb([128, T], "kT", bufs=2, dt=F32R)
                vt = sb([128, NK * 65], "vt", bufs=2, dt=F32R)
                nc.sync.dma_start(vt[:, 64::65], vone)

                # ---------- phase A: projections + norm ----------
                for j in range(NJ):
                    tsl = slice(j * TT, (j + 1) * TT)
                    pq = ps([128, TT], "pq")
                    nc.tensor.matmul(pq, wq_s, xh[:, tsl],
                                     start=True, stop=True)
                    pk = ps([128, TT], "pk")
                    nc.tensor.matmul(pk, wk_s, xh[:, tsl],
                                     start=True, stop=True)

                    # ssq for q (row 0) and k (row 32) of one PSUM bank;
                    # the winv^2 norm-weight unfold rides in the lhsT column
                    sq = sb([HD, TT], "sq", bufs=3, dt=F32R)
                    nc.scalar.square(sq, pq[0:HD, :])
                    sqk = sb([HD, TT], "sq", bufs=3, dt=F32R)
                    nc.scalar.square(sqk, pk[0:HD, :])
                    pssq = ps([1, TT], "pss", bufs=2)
                    nc.tensor.matmul(pssq, w2q_s, sq, start=True, stop=True)
                    pssk = ps([1, TT], "pss", bufs=2)
                    nc.tensor.matmul(pssk, w2k_s, sqk, start=True, stop=True)
                    lnq = sb([1, TT], "lnb", bufs=3)
                    nc.scalar.activation(lnq, pssq, AF.Ln, bias=epsb)
                    lnk = sb([1, TT], "lnb", bufs=3)
                    nc.scalar.activation(lnk, pssk, AF.Ln, bias=epsb)
                    rsq = sb([1, TT], "rs2", bufs=3)
                    nc.scalar.activation(rsq, lnq, AF.Exp, scale=-0.5,
                                         bias=ln8b)
                    rsk = sb([1, TT], "rs2", bufs=3)
                    nc.scalar.activation(rsk, lnk, AF.Exp, scale=-0.5,
                                         bias=ln8b)
                    bqb = sb([128, TT], "bqb", bufs=3)
                    nc.gpsimd.partition_broadcast(bqb, rsq)
                    nc.vector.tensor_mul(qT[:, tsl], pq, bqb)
                    bkb = sb([128, TT], "bkb", bufs=3)
                    nc.gpsimd.memset(bkb[HD:128, :], 1.0)
                    nc.gpsimd.partition_broadcast(bkb[0:HD, :], rsk)
                    nc.vector.tensor_mul(kT[:, tsl], pk, bkb)

                    # v computed directly in [t, d] layout (x^T as lhsT)
                    for u in range(TT // 128):
                        w = j * (TT // 128) + u
                        pvt = ps([128, HD], "paux")
                        nc.tensor.matmul(
                            pvt, xh[:, w * 128:(w + 1) * 128], wv_s,
                            start=True, stop=True,
                        )
                        nc.scalar.copy(vt[:, w * 65:w * 65 + HD], pvt)

                st[(b, hh)] = (qT, kT, vt)

            def emit_B(b, hh):
                qT, kT, vt = st.pop((b, hh))
                # ---------- phase B: attention ----------
                for j in range(NJ):
                    pctx = ps([65, TT], "pctx")
                    nmax = 4 * j + 3
                    for i in range(nmax + 1):
                        p = i - 4 * j
                        if p < 1:
                            off, w_ = 0, 512
                        elif p == 1:
                            off, w_ = 128, 384
                        else:
                            off, w_ = 256, 256
                        psc = ps([128, w_], "psc", bufs=2)
                        nc.tensor.matmul(
                            psc, kT[:, i * 128:(i + 1) * 128],
                            qT[:, j * TT + off:j * TT + off + w_],
                            start=True, stop=True,
                        )
                        E = sb([128, w_], "E", bufs=6, dt=F32R)
                        nc.scalar.activation(E, psc, AF.Exp, scale=0.125)
                        if p >= 0:
                            if p <= 2:
                                nc.vector.tensor_mul(
                                    E[:, 0:128], E[:, 0:128], gm_s[:, 128:256]
                                )
                            else:
                                nc.vector.tensor_mul(
                                    E[:, 0:256], E[:, 0:256], gm_s[:, 0:256]
                                )
                        nc.tensor.matmul(
                            pctx[:, off:off + w_],
                            vt[:, i * 65:(i + 1) * 65], E,
                            start=(i == 0), stop=(i == nmax),
                        )

                    # ---- unit tail: W_O in [t, o] layout, per-t 1/D ----
                    ctxs = sb([HD, TT], "ctxs", bufs=3, dt=F32R)
                    nc.vector.tensor_copy(ctxs, pctx[0:HD, :])
                    dsb = sb([65, TT], "dsb", bufs=3)
                    nc.vector.reciprocal(dsb[64:65, :], pctx[64:65, :])
                    for c in range(TT // 128):
                        # 1/D as a column: [1,128] row -> [128,1] via PE
                        pdv = ps([128, 1], "paux")
                        nc.tensor.transpose(
                            pdv, dsb[64:65, c * 128:(c + 1) * 128],
                            one_at64,
                        )
                        dvc = sb([128, 1], "dvc", bufs=4)
                        nc.vector.tensor_copy(dvc, pdv)
                        pt2 = ps([128, HD], "paux")
                        nc.tensor.matmul(
                            pt2, ctxs[:, c * 128:(c + 1) * 128], wo_s,
                            start=True, stop=True,
                        )
                        tc_ = j * (TT // 128) + c
                        nc.vector.tensor_scalar_mul(
                            obuf[b][:, tc_ * 128 + hh * HD:
                                    tc_ * 128 + hh * HD + HD],
                            pt2, dvc,
                        )

                if hh == 1:
                    # both heads of this b done -> store
                    nc.sync.dma_start(
                        out[b].rearrange("(a p) o -> p a o", p=128),
                        obuf[b].rearrange("p (a o) -> p a o", o=128),
                    )

            pairs = [(bb, hh) for bb in range(B) for hh in range(2)]
            for i in range(len(pairs) + 1):
                if i < len(pairs):
                    emit_A(*pairs[i])
                if i > 0:
                    emit_B(*pairs[i - 1])

    nc.compile()
    return nc


_CACHE = {}


def get_program(reps=1):
    key = f"nc{reps}"
    if key not in _CACHE:
        _CACHE[key] = build_program(reps)
    return _CACHE[key]


def prep_consts(W_DQ, W_UQ, W_DKV, W_UKV, W_QR, W_KR, W_O, q_norm_w, k_norm_w):
    f8 = np.float64
    wqn = q_norm_w.astype(f8)
    wkn = k_norm_w.astype(f8)
    C_q = W_UQ.astype(f8) @ W_DQ.astype(f8)                    # (64e, 64d)
    C_qr = W_QR.astype(f8) @ (wqn[:, None] * C_q)
    wq = np.ascontiguousarray(
        np.concatenate([C_q.T * wqn[None, :], C_qr.T], axis=1)
    ).astype(np.float32)
    C_kv = W_UKV.astype(f8) @ W_DKV.astype(f8)                 # (128, 64)
    wk = np.ascontiguousarray(
        np.concatenate([C_kv[:HD].T * wkn[None, :], W_KR.T.astype(f8)],
                       axis=1)
    ).astype(np.float32)
    wv = np.ascontiguousarray(C_kv[HD:].T).astype(np.float32)
    wo = np.ascontiguousarray(W_O.T).astype(np.float32)
    # ssq lhsT columns carry 1/w^2 so the norm sees raw q_c sums.
    # (a norm weight of exactly 0 would need a different path; ones here.)
    winvq = np.where(wqn != 0, 1.0 / np.where(wqn != 0, wqn, 1.0) ** 2, 0.0)
    winvk = np.where(wkn != 0, 1.0 / np.where(wkn != 0, wkn, 1.0) ** 2, 0.0)
    ident = np.eye(128, dtype=np.float32)
    gm = (np.arange(256)[None, :] - 128 >= np.arange(128)[:, None]).astype(
        np.float32
    )
    return dict(
        wq=wq, wk=wk, wv=wv, wo=wo,
        w2q=winvq.reshape(HD, 1).astype(np.float32),
        w2k=winvk.reshape(HD, 1).astype(np.float32),
        ident=ident, gm=gm,
        vone=np.ones((128, NK), np.float32),
    )


def make_in_maps(inputs, reps=1):
    x = np.asarray(inputs["x"], np.float32)
    consts = prep_consts(
        *(np.asarray(inputs[k], np.float32) for k in (
            "W_DQ", "W_UQ", "W_DKV", "W_UKV", "W_QR", "W_KR", "W_O",
            "q_norm_w", "k_norm_w"))
    )
    in_maps = []
    for c in range(NCORES):
        m = dict(consts)
        m["xs"] = np.ascontiguousarray(x[:, :, c * 128:(c + 1) * 128])
        m["rtag"] = np.zeros((1, reps), np.float32)
        in_maps.append(m)
    return in_maps


def kernel(**inputs):
    from concourse.bass_utils import run_bass_kernel_spmd

    nc = get_program()
    in_maps = make_in_maps(inputs)
    res = run_bass_kernel_spmd(nc, in_maps, core_ids=list(range(NCORES)))
    out = np.empty((B, T, D), np.float32)
    for c in range(NCORES):
        out[:, :, c * 128:(c + 1) * 128] = res.results[c]["out"]
    return out

